# revision 39
# baseline (speedup 1.0000x reference)
"""Trainium2 Bass kernel for nn_ConnectLoss.

loss = sum(relu(|x[:,j] - x[:,j-1]| - 1) * mask[:,j]) over j in [1, L).

Pure data-parallel over 8 NeuronCores: rows sharded 8192/core. Rows are
assigned partition-major (partition p owns rows [p*64, (p+1)*64) of the
shard) so every DMA reads one contiguous run per partition. Per core,
8 uniform megatiles of 8x128 rows ([128, 8, 512] SBUF tiles) stream in
on two HWDGE queues (x via sync, mask via scalar); per (sub)tile:
  DVE  tensor_tensor              d = x[:,1:] - x[:,:-1]
  ACT  activation(Abs, in-place)  d = |d|
  ACT  activation(Relu, bias=-1, in-place)  d = relu(d - 1)
  DVE  scalar_tensor_tensor       (d*1)*m with accum_out -> acc[:,col]
The kernel is DMA-bound (~33.5 MB/core at ~400 GB/s busy-rate); the
DVE/ACT loads fit underneath. The last two megatiles' compute is split
into sub-chains ([4,4] then [2,2,2,1,1]) so the serial chain left after
the final DMA lands is ~2.5 us instead of the ~16 us an 8-block chain
costs. Host sums the 8 x [128, n_acc] partials in float64.
"""
import sys

sys.path.insert(0, "/opt/trn_rl_repo")
import numpy as np

N_CORES = 8
M_ROWS = 65536
LENGTH = 512
ROWS_PER_CORE = M_ROWS // N_CORES
P = 128
BLOCKS = 8  # 128-row blocks fused per megatile

_nc_cache = None


def _build_nc(rows=ROWS_PER_CORE, length=LENGTH, blocks=BLOCKS):
    import concourse.tile as tile
    import concourse.mybir as mybir
    from concourse import bacc

    total_blocks = rows // P
    n_mega = total_blocks // blocks
    assert n_mega * blocks == total_blocks
    last = n_mega - 1

    H = blocks // 2

    def dma_slices(t):
        # Finer final-megatile DMA so its sub-chains release early.
        if t == last:
            return [(0, 2), (2, 4), (4, 6), (6, 7), (7, 8)]
        return [(0, H), (H, blocks)]

    def half_chain_slices(t, h):
        # Chain slices within half h (local coords 0..H).
        if t == last and h == 1:
            return [(0, 2), (2, 3), (3, 4)]
        if t >= last - 1:
            return [(0, 2), (2, 4)]
        return [(0, H)]

    n_acc = sum(
        len(half_chain_slices(t, h)) for t in range(n_mega) for h in (0, 1)
    )

    nc = bacc.Bacc(None)
    f32 = mybir.dt.float32
    x = nc.declare_dram_parameter("x", [rows, length], f32, isOutput=False)
    msk = nc.declare_dram_parameter("mask", [rows, length], f32, isOutput=False)
    out = nc.declare_dram_parameter("out", [P, n_acc], f32, isOutput=True)

    L1 = length - 1
    with tile.TileContext(nc) as tc:
        # A single pool (per-tag rings) keeps the TileContext epilogue
        # to one release barrier instead of four.
        with tc.tile_pool(name="all", bufs=1) as pool:
            xpool = mpool = wpool = pool
            neg1 = pool.tile([P, 1], f32, tag="neg1", bufs=1)
            nc.vector.memset(neg1[:], -1.0)
            acc = pool.tile([P, n_acc], f32, tag="acc", bufs=1)
            # Partition-major row assignment: partition p owns rows
            # [p*total_blocks, (p+1)*total_blocks) of the shard, so each
            # megatile's per-partition read is one contiguous run in
            # DRAM. The loss sums over all rows, so the order is free.
            xv = x.rearrange("(p t) m -> p t m", p=P)
            mv = msk.rearrange("(p t) m -> p t m", p=P)

            # Queue assignment: x triggers ride the Sync ring; mask
            # triggers ride the Scalar (ACT) ring inline per megatile.
            # (Alternatives measured worse: everything on one ring
            # throttles on DMAHW sem-lane recycling; SWDGE/gpsimd for
            # mask is far slower; prefetching mask triggers early
            # pollutes the compute-critical ACT queue.)
            # The LAST megatile's mask rides the Sync ring, triggered
            # right after the last x slices: on the scalar ring its
            # triggers sit behind most of the kernel's ACT compute,
            # issue only at ~87-90us, and leave the final 2MB streaming
            # alone paced by late triggers (measured 5.8us bubble). On
            # sync the triggers issue early and the data drains right
            # behind the last x at full rate.
            mt_last = mpool.tile([P, blocks, length], f32, tag="mt7", bufs=1)
            col = 0
            for t in range(n_mega):
                r0 = t * blocks
                xt = xpool.tile([P, blocks, length], f32, tag="xt", bufs=5)
                for a, b in dma_slices(t):
                    nc.sync.dma_start(xt[:, a:b], xv[:, r0 + a : r0 + b, :])
                if t == last:
                    mt = mt_last
                    for a, b in [(0, H), (H, blocks)]:
                        nc.sync.dma_start(
                            mt[:, a:b], mv[:, r0 + a : r0 + b, :]
                        )
                else:
                    mt = mpool.tile([P, blocks, length], f32, tag="mt", bufs=4)
                    for a, b in [(0, H), (H, blocks)]:
                        nc.scalar.dma_start(
                            mt[:, a:b], mv[:, r0 + a : r0 + b, :]
                        )
                # Chains run at half-megatile granularity with a
                # half-sized d workspace: d bufs sets the
                # software-pipelining (hoist) depth (bufs=1 serializes
                # chains into 16us/megatile, measured; ~2.5 megatiles
                # of hoist depth lets compute track the 10.4us/megatile
                # stream), and half-sized d bufs fit 5 deep alongside
                # 5-deep x/m rings in the 207.8KB SBUF budget.
                for h in (0, 1):
                    hb = h * H
                    d = wpool.tile([P, H, length], f32, tag="d", bufs=5)
                    for la, lb in half_chain_slices(t, h):
                        a, b = hb + la, hb + lb
                        nc.vector.tensor_tensor(
                            d[:, la:lb, 1:length],
                            xt[:, a:b, 1:length],
                            xt[:, a:b, 0:L1],
                            mybir.AluOpType.subtract,
                        )
                        nc.scalar.activation(
                            d[:, la:lb, 1:length],
                            d[:, la:lb, 1:length],
                            mybir.ActivationFunctionType.Abs,
                        )
                        nc.scalar.activation(
                            d[:, la:lb, 1:length],
                            d[:, la:lb, 1:length],
                            mybir.ActivationFunctionType.Relu,
                            bias=neg1[:, 0:1],
                            scale=1.0,
                        )
                        # Main-out is garbage; aim it at the dead x
                        # slice (fully consumed by the subtract)
                        # instead of a dedicated junk tile.
                        nc.vector.scalar_tensor_tensor(
                            xt[:, a:b, 1:length],
                            d[:, la:lb, 1:length],
                            1.0,
                            mt[:, a:b, 1:length],
                            op0=mybir.AluOpType.mult,
                            op1=mybir.AluOpType.mult,
                            accum_out=acc[:, col : col + 1],
                        )
                        col += 1
            nc.sync.dma_start(out[:], acc[:])
    nc.compile()
    return nc


def _get_nc():
    global _nc_cache
    if _nc_cache is None:
        _nc_cache = _build_nc()
    return _nc_cache


def _finish(outs) -> np.ndarray:
    o = np.stack(outs).astype(np.float64)
    return np.asarray(o.sum(), dtype=np.float32)


def run_spmd(x, mask, trace: bool = False):
    """Returns (loss ndarray, BassKernelResults)."""
    from concourse.bass_utils import run_bass_kernel_spmd

    x = np.ascontiguousarray(np.asarray(x, dtype=np.float32))
    mask = np.ascontiguousarray(np.asarray(mask, dtype=np.float32))
    assert x.shape == (M_ROWS, LENGTH) and mask.shape == (M_ROWS, LENGTH)

    in_maps = [
        {
            "x": x[i * ROWS_PER_CORE : (i + 1) * ROWS_PER_CORE],
            "mask": mask[i * ROWS_PER_CORE : (i + 1) * ROWS_PER_CORE],
        }
        for i in range(N_CORES)
    ]
    res = run_bass_kernel_spmd(
        _get_nc(), in_maps, list(range(N_CORES)), trace=trace
    )
    loss = _finish([r["out"] for r in res.results])
    return loss, res


def kernel(x, mask) -> np.ndarray:
    loss, _ = run_spmd(x, mask, trace=False)
    return loss
